# revision 4
# baseline (speedup 1.0000x reference)
"""Trainium2 Bass kernel for nn_FNORCF1d — power-sum CFT restructure.

Sharding: data-parallel over batch (core b = batch element b). One tiny
AllGather (15us, vs 28us AllReduce) + local GpSimd partition-max per layer
for the CFT per-segment min/max.

Key restructurings vs the original kernel (sim: 748us -> 275us):
  - CFT via raw power sums: rec[c,s] = tanh(sum_k e_k[c,s] * S_k[c,s]) where
    S_k = mean((a h + b)^k) = (1/LS) sum_j C(k,j) a^j b^{k-j} P_j and
    P_j = sum_l h^j over the segment. P_j depends only on h, so the heavy
    elementwise phase is independent of the collective; only a tiny per-(c,s)
    binomial combine (a 36x36 matmul whose operand is built from b-powers on
    per-partition scalars) waits on it. Combine+gate-bias run per segment.
  - h, xfno, and the conv/gate/fc1 matmuls carried in bf16 (~6e-3 end-to-end,
    budget 2e-2); h -> hT via per-segment XBAR DMA block-transposes (no PE
    transposes, no PSUM->SBUF copies).
  - min/max read bf16 h directly via tensor_scalar(op1=max, accum_out) at the
    DVE 4x rate; products as tensor_tensor (2x) + tensor_scalar-accum sums
    (4x) -- scalar_tensor_tensor/tensor_reduce have no DVE perf modes.
  - engine split: squares on Act, odd products+sums+minmax+updates on DVE,
    p6/p8 products on GpSimd (no STT/PSUM/X-reduce allowed there).
  - P1 rides the gate-update STT accumulators; gate at 1024-wide pairs.
  - tail gelu via erf (same act-table set as sigmoid: no table-load ping-pong
    while the last layer's sigmoids drain) + a sigmoid-set preload dummy.
  - xt packed [32, 512] (one 64KB f32r DMA, K=32 zero-masked fc0 stationary);
    y via z-stationary matmuls -> [128, 64] on partitions -> one DMA.
"""

import functools
import math
import os
from contextlib import ExitStack

import numpy as np
from ml_dtypes import bfloat16

import concourse.bass as bass
import concourse.bacc as bacc
import concourse.mybir as mybir
import concourse.tile as tile
from concourse.bass_utils import run_bass_kernel_spmd

F32 = mybir.dt.float32
F32R = mybir.dt.float32r
BF16 = mybir.dt.bfloat16
AF = mybir.ActivationFunctionType
ALU = mybir.AluOpType
AX = mybir.AxisListType

B, L, W, MODES, NL, SEG, CM, H1 = 8, 8192, 128, 32, 4, 4, 8, 128
LS = L // SEG           # 2048
NJ = L // 512           # 16 chunks of 512
NC128 = L // 128        # 64 chunks of 128
K2 = 2 * MODES          # 64 interleaved (re, im) rows
NP = 9                  # powers 0..8


def _cheb2poly():
    """tmat[n, k]: T_n(x) = sum_k tmat[n,k] x^k, n,k in 0..8."""
    t = np.zeros((NP, NP))
    t[0, 0] = 1.0
    t[1, 1] = 1.0
    for n in range(2, NP):
        t[n, 1:] += 2.0 * t[n - 1, :-1]
        t[n, :] -= t[n - 2, :]
    return t


def _host_consts():
    lg = np.arange(L)
    kg = np.arange(MODES)
    ang = 2.0 * np.pi * np.outer(lg, kg) / L          # [L, 32]
    fall = np.zeros((L, 128), np.float32)
    fall[:, :MODES] = np.cos(ang)
    fall[:, MODES:K2] = -np.sin(ang)
    fcat = fall.reshape(NC128, 128, 128).transpose(1, 0, 2).reshape(128, NC128 * 128)
    coefk = (np.where(kg == 0, 1.0, 2.0) / L).astype(np.float32)
    minv = np.zeros((128, L), np.float32)
    minv[0:K2:2] = coefk[:, None] * np.cos(ang.T)
    minv[1:K2:2] = -coefk[:, None] * np.sin(ang.T)

    # binomial diagonal masks: block d has C(j+d, j) at (row s*9+j+d, col s*9+j)
    hcd = np.zeros((128, NP * 128), np.float32)
    for d in range(NP):
        for s in range(SEG):
            for j in range(NP - d):
                hcd[s * NP + j + d, d * 128 + s * NP + j] = math.comb(j + d, j)
    return {
        "fcat": fcat.astype(bfloat16),
        "minv": minv.astype(bfloat16),
        "ident_f": np.eye(128, dtype=np.float32),
        "ident_b": np.eye(128).astype(bfloat16),
        "ones128": np.ones((128, 128), np.float32),
        "hcd": hcd.astype(bfloat16),
    }


def _host_weights(inputs):
    spec = np.empty((NL, 128, MODES * 256), np.float32)
    for i in range(NL):
        for k in range(MODES):
            spec[i, :, k * 256: k * 256 + 128] = inputs["spec_wr"][i][:, :, k]
            spec[i, :, k * 256 + 128: (k + 1) * 256] = inputs["spec_wi"][i][:, :, k]

    # e-fold: arg[c,s] = sum_k e_k[c,s] S_k[c,s]
    # coeffs C_m = mean(T1*T_m) = mean(T_{m+1} + T_{|m-1|})/2; d_n = weights on
    # mean(T_n); e_k = sum_n d_n tmat[n,k] / LS  (1/LS folded in).
    tmat = _cheb2poly()
    cb = inputs["cheb_w"].mean(-1)                    # [NL, SEG, CM, W]
    d = np.zeros((NL, SEG, NP, W))
    d[:, :, 1] += cb[:, :, 0]
    for m in range(1, CM):
        d[:, :, m + 1] += cb[:, :, m] / 2
        d[:, :, abs(m - 1)] += cb[:, :, m] / 2
    e = np.einsum('lsnc,nk->lskc', d, tmat) / LS      # [NL, SEG, 9, W]
    eT = np.zeros((128, NL * 128), np.float32)        # rows (s*9+k), cols l*128+c
    for li in range(NL):
        for s in range(SEG):
            for k in range(NP):
                eT[s * NP + k, li * 128: (li + 1) * 128] = e[li, s, k, :]

    fc0w2 = np.zeros((2 * NJ, NJ * W), np.float32)  # block j: rows 2j:2j+2
    for j in range(NJ):
        fc0w2[2 * j:2 * j + 2, j * W:(j + 1) * W] = inputs["fc0_w"]
    fc2wS = inputs["fc2_w"].astype(np.float32)        # [128, 1]
    fc2bS = np.full((128, 1), float(inputs["fc2_b"][0]), np.float32)

    return {
        "spec": spec.astype(bfloat16),
        "convw": np.concatenate(list(inputs["conv_w"]), axis=1).astype(bfloat16),
        "convb": inputs["conv_b"].T.astype(np.float32),
        "gwa": np.concatenate([inputs["gate_w"][i][:W] for i in range(NL)], axis=1).astype(bfloat16),
        "gwb": np.concatenate([inputs["gate_w"][i][W:] for i in range(NL)], axis=1).astype(np.float32),
        "gateb": inputs["gate_b"].T.astype(np.float32),
        "eT": eT,
        "fc0w2": fc0w2,
        "fc0b": inputs["fc0_b"].reshape(W, 1).astype(np.float32),
        "fc1w": inputs["fc1_w"].astype(bfloat16),
        "fc1b": (inputs["fc1_b"] / np.sqrt(2.0)).reshape(H1, 1).astype(np.float32),
        "fc2wS": fc2wS,
        "fc2bS": fc2bS,
    }


_SPECS = {
    "xt": ((2 * NJ, 512), F32R),
    "fcat": ((128, NC128 * 128), BF16),
    "minv": ((128, L), BF16),
    "ident_f": ((128, 128), F32),
    "ident_b": ((128, 128), BF16),
    "ones128": ((128, 128), F32),
    "hcd": ((128, NP * 128), BF16),
    "spec": ((NL, 128, MODES * 256), BF16),
    "convw": ((128, NL * 128), BF16),
    "convb": ((128, NL), F32),
    "gwa": ((128, NL * 128), BF16),
    "gwb": ((128, NL * 128), F32),
    "gateb": ((128, NL), F32),
    "eT": ((128, NL * 128), F32),
    "fc0w2": ((2 * NJ, NJ * W), F32R),
    "fc0b": ((128, 1), F32),
    "fc1w": ((128, 128), BF16),
    "fc1b": ((128, 1), F32),
    "fc2wS": ((128, 1), F32),
    "fc2bS": ((128, 1), F32),
}

# load order matters: first items unblock fc0 + layer-0 FNO
_CONST_NAMES = ["fc0w2", "fc0b", "ident_f", "fcat", "ident_b", "minv",
                "convw", "convb", "hcd", "eT", "ones128", "gwa", "gwb", "gateb",
                "fc1w", "fc1b", "fc2wS", "fc2bS"]
_CONST_EARLY = ["fc0w2", "fc0b", "ident_f", "fcat", "ident_b"]


def _emit(tc, ap, ctx):
    lvl = int(os.environ.get("KBISECT", "99"))
    nc = tc.nc
    ec = ctx.enter_context

    cpool = ec(tc.tile_pool(name="const", bufs=1))
    spool = ec(tc.tile_pool(name="spec", bufs=2))
    state = ec(tc.tile_pool(name="state", bufs=1))
    cft = ec(tc.tile_pool(name="cft", bufs=2))
    small = ec(tc.tile_pool(name="small", bufs=2))
    gpool = ec(tc.tile_pool(name="gate", bufs=2))
    dpool = ec(tc.tile_pool(name="dram", bufs=4, space="DRAM"))
    ptr = ec(tc.tile_pool(name="ptr", bufs=2, space="PSUM"))
    psmall = ec(tc.tile_pool(name="psmall", bufs=2, space="PSUM"))
    pbig = ec(tc.tile_pool(name="pbig", bufs=2, space="PSUM"))

    tiles = {}
    xt32_holder = []

    def _load(names):
        for name in names:
            sh, dt_ = _SPECS[name]
            t = cpool.tile(list(sh), dt_, tag=name, name=f"c_{name}")
            nc.sync.dma_start(t[:], ap[name][:])
            tiles[name] = t
    xt32_pre = small.tile([2 * NJ, 512], F32R, tag="xt32", bufs=1)
    nc.sync.dma_start(xt32_pre[:], ap["xt"][:])
    xt32_holder.append(xt32_pre)
    _load(_CONST_EARLY)

    h = state.tile([128, L], BF16, tag="h")
    xfno = state.tile([128, L], BF16, tag="xfno")
    hT = state.tile([128, L], BF16, tag="hT")
    accP = state.tile([128, 128], F32, tag="accP")       # cols (s*9+j), j=0..8
    accP1c2 = state.tile([128, NJ // 2], F32, tag="accP1c")  # gate-part P1
    accP1g = state.tile([128, NJ // 2], F32, tag="accP1g")  # fc0 P1
    arow = state.tile([128, SEG * NP], F32, tag="arow")  # row0: a^j at (s*9+j)
    brow = state.tile([128, 128], F32, tag="brow")       # row0: b at (s*9+k)
    bdcol = state.tile([128, NP], F32, tag="bdcol")      # col d: b^d rows (s,k)
    mbt = state.tile([128, 128], F32, tag="mbt")
    qTsb = state.tile([128, 128], F32, tag="qTsb")
    omT_sb = state.tile([128, 128], BF16, tag="omT_sb")

    # one-time zero/const initialization
    nc.gpsimd.memset(arow[:], 0.0)
    nc.gpsimd.memset(brow[:], 0.0)
    nc.gpsimd.memset(omT_sb[:], 0.0)
    nc.gpsimd.memset(bdcol[:], 1.0)          # col 0 stays 1; cols 1.. rebuilt
    nc.gpsimd.memset(accP[:], float(LS))     # (s,0) cols stay LS; rest rebuilt
    arv = arow[0:1, 0:SEG * NP].rearrange("one (s j) -> one s j", j=NP)
    nc.gpsimd.memset(arv[:, :, 0:1], 1.0)    # a^0 = 1

    spec_t = [None] * NL
    spec_t[0] = spool.tile([128, MODES * 256], BF16, tag="spec", name="spec0")
    nc.sync.dma_start(spec_t[0][:], ap["spec"][0])
    _load([n for n in _CONST_NAMES if n not in _CONST_EARLY])
    fcat, minv = tiles["fcat"], tiles["minv"]
    ident_f, ident_b = tiles["ident_f"], tiles["ident_b"]
    ones128, hcd, eT = tiles["ones128"], tiles["hcd"], tiles["eT"]
    convw, convb = tiles["convw"], tiles["convb"]
    gwa, gwb, gateb = tiles["gwa"], tiles["gwb"], tiles["gateb"]
    fc0w2, fc0b = tiles["fc0w2"], tiles["fc0b"]
    fc1w, fc1b = tiles["fc1w"], tiles["fc1b"]
    fc2wS, fc2bS = tiles["fc2wS"], tiles["fc2bS"]


    # ---- fc0: h = fc0_w.T @ x (K=2 contraction per row pair) + P1 accum ----
    xt32 = xt32_holder[0]
    for j2 in range(NJ // 2):
        js = slice(j2 * 1024, (j2 + 1) * 1024)
        p = pbig.tile([128, 1024], F32, tag="xfp", bufs=1)
        for hh in range(2):
            j = 2 * j2 + hh
            nc.tensor.matmul(p[:, hh * 512:(hh + 1) * 512],
                             fc0w2[:, j * W:(j + 1) * W],
                             xt32[:], start=True, stop=True)
        nc.scalar.activation(h[:, js], p[:], AF.Identity, bias=fc0b[:],
                             accum_out=accP1g[:, j2:j2 + 1])

    nlayers = 0 if lvl < 2 else NL
    for i in range(nlayers):
        last = i == NL - 1
        # ---- h -> hT via per-segment DMA block-transposes (XBAR) ----
        for s in range(SEG):
            hTv = hT[:, s * LS:(s + 1) * LS].rearrange("p (c j) -> p c j", j=128)
            nc.sync.dma_start_transpose(hTv, h[:, s * LS:(s + 1) * LS])

        # ---- min/max direct from bf16 h segments (DVE 4x) -> AllGather ----
        catmm = small.tile([128, 8], F32, tag="catmm")
        for s in range(SEG):
            hs = h[:, s * LS:(s + 1) * LS]
            mmscr = cft.tile([128, LS], BF16, tag="scr", bufs=2)
            nc.vector.tensor_scalar(mmscr[:], hs, -1.0, None, ALU.mult, ALU.max,
                                    accum_out=catmm[:, s:s + 1])
            nc.vector.tensor_scalar(mmscr[:], hs, 1.0, None, ALU.mult, ALU.max,
                                    accum_out=catmm[:, SEG + s:SEG + s + 1])
        redrow = small.tile([1, 2 * SEG], F32, tag="redrow")
        nc.gpsimd.tensor_reduce(redrow[:], catmm[:], AX.C, ALU.max)
        ccin = dpool.tile([1, 2 * SEG], F32, tag="ccin")
        ccout = dpool.tile([B, 2 * SEG], F32, tag="ccout")
        nc.gpsimd.dma_start(ccin[:], redrow[:])
        nc.gpsimd.collective_compute(
            "AllGather", ALU.bypass,
            ins=[ccin[:].opt()], outs=[ccout[:].opt()],
            replica_groups=[list(range(B))],
        )

        # ---- forward DFT + mode mix (PE; small copies on Pool) ----
        phfT = psmall.tile([128, 128], F32, tag="sm", bufs=2)
        for c in range(NC128):
            nc.tensor.matmul(phfT[:], fcat[:, c * 128:(c + 1) * 128],
                             hT[:, c * 128:(c + 1) * 128],
                             start=(c == 0), stop=(c == NC128 - 1))
        hfT_sb = small.tile([128, 128], BF16, tag="hfT_sb", bufs=1)
        nc.vector.tensor_copy(hfT_sb[0:K2, :], phfT[0:K2, :])
        phf = psmall.tile([128, 128], BF16, tag="sm", bufs=2)
        nc.tensor.transpose(phf[:], hfT_sb[:], ident_b[:])
        rhs1 = small.tile([128, K2], BF16, tag="rhs1", bufs=1)
        rhs2 = small.tile([128, K2], BF16, tag="rhs2", bufs=1)
        r1v = rhs1[:].rearrange("p (k two) -> p k two", two=2)
        r2v = rhs2[:].rearrange("p (k two) -> p k two", two=2)
        hrv = phf[:, 0:MODES].rearrange("p k -> p k ()")
        hiv = phf[:, MODES:K2].rearrange("p k -> p k ()")
        nc.vector.tensor_copy(r1v[:, :, 0:1], hrv)
        nc.vector.tensor_copy(r1v[:, :, 1:2], hiv)
        nc.vector.tensor_scalar(r2v[:, :, 0:1], hiv, -1.0, None, ALU.mult)
        nc.vector.tensor_copy(r2v[:, :, 1:2], hrv)

        pom = psmall.tile([128, K2], F32, tag="sm", bufs=2)
        for k in range(MODES):
            nc.tensor.matmul(pom[:, 2 * k:2 * k + 2],
                             spec_t[i][:, k * 256:k * 256 + 128],
                             rhs1[:, 2 * k:2 * k + 2], start=True, stop=False)
            nc.tensor.matmul(pom[:, 2 * k:2 * k + 2],
                             spec_t[i][:, k * 256 + 128:(k + 1) * 256],
                             rhs2[:, 2 * k:2 * k + 2], start=False, stop=True)
        om_sb = small.tile([128, 128], BF16, tag="om_sb", bufs=1)
        nc.vector.tensor_copy(om_sb[:, 0:K2], pom[:])
        pomT = psmall.tile([128, 128], BF16, tag="sm", bufs=2)
        nc.tensor.transpose(pomT[:], om_sb[:], ident_b[:])
        nc.vector.tensor_copy(omT_sb[0:K2, :], pomT[0:K2, :])

        if i + 1 < NL:
            spec_t[i + 1] = spool.tile([128, MODES * 256], BF16, tag="spec",
                                       name=f"spec{i + 1}")
            nc.sync.dma_start(spec_t[i + 1][:], ap["spec"][i + 1])

        if lvl < 4:
            continue
        # ---- power sums P_1..P_8 per segment ----
        aPv = accP[0:128, 0:SEG * NP].rearrange("p (s j) -> p s j", j=NP)
        if i == 0:
            a1v = accP1g[:].rearrange("p (s f) -> p s f", f=(NJ // 2) // SEG)
            nc.vector.tensor_reduce(aPv[:, :, 1:2], a1v, AX.X, ALU.add)
        else:
            a1c = accP1c2[:].rearrange("p (s f) -> p s f", f=(NJ // 2) // SEG)
            nc.vector.tensor_reduce(aPv[:, :, 1:2], a1c, AX.X, ALU.add)
        for s in range(SEG):
            hseg = h[:, s * LS:(s + 1) * LS]
            sj = s * NP
            h2 = cft.tile([128, LS], BF16, tag="h2", bufs=1)
            nc.scalar.activation(h2[:], hseg, AF.Square,
                                 accum_out=accP[:, sj + 2:sj + 3])
            h3 = cft.tile([128, LS], BF16, tag="h3", bufs=2)
            nc.vector.tensor_tensor(h3[:], h2[:], hseg, ALU.mult)
            h4 = cft.tile([128, LS], BF16, tag="h4", bufs=1)
            nc.scalar.activation(h4[:], h2[:], AF.Square,
                                 accum_out=accP[:, sj + 4:sj + 5])
            scr3 = cft.tile([128, LS], BF16, tag="scr", bufs=2)
            nc.vector.tensor_scalar(scr3[:], h3[:], 1.0, None, ALU.mult, ALU.add,
                                    accum_out=accP[:, sj + 3:sj + 4])
            scr5 = cft.tile([128, LS], BF16, tag="scr", bufs=2)
            nc.vector.tensor_tensor(scr5[:], h2[:], h3[:], ALU.mult)
            nc.vector.tensor_scalar(scr5[:], scr5[:], 1.0, None, ALU.mult, ALU.add,
                                    accum_out=accP[:, sj + 5:sj + 6])
            scr6 = cft.tile([128, LS], BF16, tag="scrp", bufs=1)
            nc.gpsimd.tensor_tensor(scr6[:], h3[:], h3[:], ALU.mult)
            nc.vector.tensor_scalar(scr6[:], scr6[:], 1.0, None, ALU.mult, ALU.add,
                                    accum_out=accP[:, sj + 6:sj + 7])
            scr7 = cft.tile([128, LS], BF16, tag="scr", bufs=2)
            nc.vector.tensor_tensor(scr7[:], h3[:], h4[:], ALU.mult)
            nc.vector.tensor_scalar(scr7[:], scr7[:], 1.0, None, ALU.mult, ALU.add,
                                    accum_out=accP[:, sj + 7:sj + 8])
            scr8 = cft.tile([128, LS], BF16, tag="scrp", bufs=1)
            nc.gpsimd.tensor_tensor(scr8[:], h4[:], h4[:], ALU.mult)
            nc.vector.tensor_scalar(scr8[:], scr8[:], 1.0, None, ALU.mult, ALU.add,
                                    accum_out=accP[:, sj + 8:sj + 9])

        if lvl < 5:
            continue
        # ---- spectral + conv -> gelu -> x_fno ----
        for j2 in range(NJ // 2):
            js = slice(j2 * 1024, (j2 + 1) * 1024)
            p = pbig.tile([128, 1024], F32, tag="xfp", bufs=1)
            for hh in range(2):
                hs = slice(j2 * 1024 + hh * 512, j2 * 1024 + (hh + 1) * 512)
                nc.tensor.matmul(p[:, hh * 512:(hh + 1) * 512], omT_sb[:],
                                 minv[:, hs], start=True, stop=False)
                nc.tensor.matmul(p[:, hh * 512:(hh + 1) * 512],
                                 convw[:, i * 128:(i + 1) * 128], h[:, hs],
                                 start=False, stop=True)
            nc.scalar.activation(xfno[:, js], p[:], AF.Gelu, bias=convb[:, i:i + 1])

        sgw = small.tile([1, 1], F32, tag="sgw")
        nc.scalar.activation(sgw[:], xfno[0:1, L - 1:L], AF.Sigmoid)

        if lvl < 6:
            continue
        # ---- combine (Pool smalls wait on the AllGather) ----
        gat = small.tile([B, 2 * SEG], F32, tag="gat")
        abrow = small.tile([1, 2 * SEG], F32, tag="abrow")
        with tc.high_priority():
            nc.sync.dma_start(gat[:], ccout[:])
            nc.gpsimd.tensor_reduce(abrow[:], gat[:], AX.C, ALU.max)
        hp = tc.high_priority()
        hp.__enter__()
        negd = small.tile([1, SEG], F32, tag="negd")
        nc.gpsimd.tensor_tensor(negd[:], abrow[:, 0:SEG], abrow[:, SEG:2 * SEG], ALU.add)
        inv = small.tile([1, SEG], F32, tag="invd")
        nc.vector.reciprocal(inv[:], negd[:])          # 1/(mx-mn)
        a4 = small.tile([1, SEG], F32, tag="a4")
        nc.gpsimd.tensor_scalar(a4[:], inv[:], 2.0, None, ALU.mult)
        m1 = small.tile([1, SEG], F32, tag="m1")
        nc.gpsimd.tensor_tensor(m1[:], abrow[:, 0:SEG], inv[:], ALU.mult)
        b4 = small.tile([1, SEG], F32, tag="b4")
        nc.gpsimd.tensor_scalar(b4[:], m1[:], 2.0, -1.0, ALU.mult, ALU.add)
        # arow row0: a^j; brow row0: b at every (s,k)
        for j in range(1, NP):
            nc.gpsimd.tensor_tensor(arv[:, :, j:j + 1], arv[:, :, j - 1:j],
                                    a4[:].rearrange("one s -> one s ()"), ALU.mult)
        brv = brow[0:1, 0:SEG * NP].rearrange("one (s k) -> one s k", k=NP)
        for k in range(NP):
            nc.gpsimd.tensor_copy(brv[:, :, k:k + 1],
                                  b4[:].rearrange("one s -> one s ()"))
        pcol = psmall.tile([128, 128], F32, tag="sm", bufs=2)
        nc.tensor.transpose(pcol[:], brow[:], ident_f[:])
        bcol = small.tile([128, 1], F32, tag="bcol")
        nc.vector.tensor_copy(bcol[:], pcol[:, 0:1])
        nc.gpsimd.tensor_copy(bdcol[:, 1:2], bcol[:])
        nc.gpsimd.tensor_tensor(bdcol[:, 2:3], bcol[:], bcol[:], ALU.mult)
        nc.gpsimd.tensor_tensor(bdcol[:, 3:4], bdcol[:, 1:2], bdcol[:, 2:3], ALU.mult)
        nc.gpsimd.tensor_tensor(bdcol[:, 4:5], bdcol[:, 2:3], bdcol[:, 2:3], ALU.mult)
        nc.gpsimd.tensor_tensor(bdcol[:, 5:6], bdcol[:, 2:3], bdcol[:, 3:4], ALU.mult)
        nc.gpsimd.tensor_tensor(bdcol[:, 6:7], bdcol[:, 3:4], bdcol[:, 3:4], ALU.mult)
        nc.gpsimd.tensor_tensor(bdcol[:, 7:8], bdcol[:, 3:4], bdcol[:, 4:5], ALU.mult)
        nc.gpsimd.tensor_tensor(bdcol[:, 8:9], bdcol[:, 4:5], bdcol[:, 4:5], ALU.mult)
        mb2 = small.tile([128, 128], F32, tag="mb2", bufs=1)
        mb3 = small.tile([128, 128], F32, tag="mb3", bufs=1)
        nc.gpsimd.tensor_copy(mbt[:], hcd[:, 0:128])
        for d in (1, 2):
            nc.vector.scalar_tensor_tensor(mbt[:], hcd[:, d * 128:(d + 1) * 128],
                                           bdcol[:, d:d + 1], mbt[:],
                                           ALU.mult, ALU.add)
        nc.vector.tensor_scalar(mb2[:], hcd[:, 3 * 128:4 * 128], bdcol[:, 3:4],
                                None, ALU.mult)
        for d in (4, 5):
            nc.vector.scalar_tensor_tensor(mb2[:], hcd[:, d * 128:(d + 1) * 128],
                                           bdcol[:, d:d + 1], mb2[:],
                                           ALU.mult, ALU.add)
        nc.vector.tensor_scalar(mb3[:], hcd[:, 6 * 128:7 * 128], bdcol[:, 6:7],
                                None, ALU.mult)
        for d in (7, 8):
            nc.vector.scalar_tensor_tensor(mb3[:], hcd[:, d * 128:(d + 1) * 128],
                                           bdcol[:, d:d + 1], mb3[:],
                                           ALU.mult, ALU.add)
        nc.vector.tensor_tensor(mb2[:], mb2[:], mb3[:], ALU.add)
        nc.vector.tensor_tensor(mbt[:], mbt[:], mb2[:], ALU.add)
        pqT = psmall.tile([128, 128], F32, tag="sm", bufs=2)
        nc.tensor.matmul(pqT[:], mbt[:], eT[:, i * 128:(i + 1) * 128],
                         start=True, stop=True)
        nc.vector.tensor_copy(qTsb[:], pqT[:])
        pQ = psmall.tile([128, 128], F32, tag="sm", bufs=2)
        nc.tensor.transpose(pQ[:], qTsb[:], ident_f[:])
        pA = psmall.tile([128, SEG * NP], F32, tag="sm", bufs=2)
        nc.tensor.matmul(pA[:], ones128[:], arow[:, 0:SEG * NP],
                         start=True, stop=True)
        # per-segment combine tail: rec[s] only needs segment-s power sums,
        # so early segments' gate pairs start while later products finish
        u = small.tile([128, SEG * NP], F32, tag="u")
        parg = small.tile([128, SEG], F32, tag="parg")
        rec = small.tile([128, SEG], F32, tag="rec")
        biasg = small.tile([128, SEG], F32, tag="biasg")
        for s in range(SEG):
            sl = slice(s * NP, (s + 1) * NP)
            nc.vector.tensor_tensor(u[:, sl], accP[:, sl], pA[:, sl], ALU.mult)
            nc.vector.tensor_tensor(u[:, sl], u[:, sl], pQ[:, sl], ALU.mult)
            nc.vector.tensor_reduce(
                parg[:, s:s + 1],
                u[:, sl].rearrange("p (one j) -> p one j", one=1), AX.X, ALU.add)
            nc.scalar.activation(rec[:, s:s + 1], parg[:, s:s + 1], AF.Tanh)
            pgs = psmall.tile([128, 1], F32, tag="sm", bufs=2, name=f"pgs{i}_{s}")
            nc.tensor.matmul(pgs[:], gwb[:, i * 128:(i + 1) * 128], rec[:, s:s + 1],
                             start=True, stop=True)
            nc.vector.tensor_scalar(biasg[:, s:s + 1], pgs[:], gateb[:, i:i + 1],
                                    None, ALU.add)
        hp.__exit__(None, None, None)

        if lvl < 7:
            continue

        for j2 in range(NJ // 2):
            js = slice(j2 * 1024, (j2 + 1) * 1024)
            s = j2 // 2
            pg = pbig.tile([128, 1024], F32, tag="gate", bufs=2)
            nc.tensor.matmul(pg[:, 0:512], gwa[:, i * 128:(i + 1) * 128],
                             xfno[:, j2 * 1024:j2 * 1024 + 512], start=True, stop=True)
            nc.tensor.matmul(pg[:, 512:1024], gwa[:, i * 128:(i + 1) * 128],
                             xfno[:, j2 * 1024 + 512:(j2 + 1) * 1024],
                             start=True, stop=True)
            gchunk = gpool.tile([128, 1024], BF16, tag="g")
            nc.scalar.activation(gchunk[:], pg[:], AF.Sigmoid, bias=biasg[:, s:s + 1])
            nc.vector.scalar_tensor_tensor(
                h[:, js], gchunk[:], rec[:, s:s + 1], xfno[:, js],
                ALU.mult, ALU.add,
                accum_out=None if last else accP1c2[:, j2:j2 + 1])

    if lvl < 9:
        return
    # ---- tail: fc1 -> gelu -> fc2 -> y ----
    # gelu via erf (stays in the sigmoid act-table set: no table reloads while
    # the last layer's sigmoids drain): gelu(x) = (0.5 erf(x/sqrt2) + 0.5) * x
    z = state.tile([128, L], F32, tag="ztail")
    for j2 in range(NJ // 2):
        js = slice(j2 * 1024, (j2 + 1) * 1024)
        p = pbig.tile([128, 1024], F32, tag="gate", bufs=2)
        nc.tensor.matmul(p[:, 0:512], fc1w[:], h[:, j2 * 1024:j2 * 1024 + 512],
                         start=True, stop=True)
        nc.tensor.matmul(p[:, 512:1024], fc1w[:],
                         h[:, j2 * 1024 + 512:(j2 + 1) * 1024],
                         start=True, stop=True)
        ec_ = gpool.tile([128, 1024], F32, tag="gr", bufs=1)
        nc.scalar.activation(ec_[:], p[:], AF.Erf, bias=fc1b[:],
                             scale=float(1.0 / np.sqrt(2.0)))
        nc.vector.tensor_scalar(ec_[:], ec_[:], 0.5, 0.5, ALU.mult, ALU.add)
        nc.vector.tensor_tensor(z[:, js], ec_[:], p[:], ALU.mult)
    # y on partitions: out[p, c] = sum_k z[k, c*128+p] fc2w[k] -> y[c*128+p]
    py = psmall.tile([128, 128], F32, tag="sm", bufs=2)
    yc = small.tile([128, NC128], F32, tag="yc", bufs=1)
    for g in range(NC128 // 32):
        for t in range(32):
            c = g * 32 + t
            nc.tensor.matmul(py[:, t:t + 1], z[:, c * 128:(c + 1) * 128],
                             fc2wS[:, 0:1], start=True, stop=True)
        nc.scalar.activation(yc[:, g * 32:(g + 1) * 32], py[:, 0:32],
                             AF.Identity, bias=fc2bS[:])
    nc.sync.dma_start(ap["y"][:], yc[:])


@functools.lru_cache(maxsize=1)
def _build():
    nc = bacc.Bacc("TRN2", target_bir_lowering=False, debug=False, num_devices=B)
    ap = {}
    for name, (shape, dt_) in _SPECS.items():
        ap[name] = nc.dram_tensor(name, list(shape), dt_, kind="ExternalInput").ap()
    ap["y"] = nc.dram_tensor("y", [128, NC128], F32, kind="ExternalOutput").ap()
    with tile.TileContext(nc) as tc:
        with ExitStack() as ctx:
            _emit(tc, ap, ctx)
    nc.compile()
    return nc


def kernel(**inputs):
    inputs = {k: np.asarray(v) for k, v in inputs.items()}
    consts = _host_consts()
    weights = _host_weights(inputs)
    nc = _build()

    shared = {}
    for name in _SPECS:
        if name == "xt":
            continue
        src = consts.get(name)
        if src is None:
            src = weights[name]
        shared[name] = np.ascontiguousarray(src)
    in_maps = []
    for b in range(B):
        m = dict(shared)
        xv = inputs["x"][b].T.astype(np.float32).reshape(2, NJ, 512)
        m["xt"] = np.ascontiguousarray(xv.transpose(1, 0, 2).reshape(2 * NJ, 512))
        in_maps.append(m)

    res = run_bass_kernel_spmd(nc, in_maps, list(range(B)))
    out = np.stack([res.results[b]["y"].T.reshape(L, 1) for b in range(B)])
    return out.astype(np.float32)


if __name__ == "__main__":
    import reference

    inputs = reference.setup_inputs()
    out = kernel(**{k: np.asarray(v) for k, v in inputs.items()})
    print(out.shape, np.abs(out).max())


# revision 6
# speedup vs baseline: 1.0950x; 1.0950x over previous
"""Trainium2 Bass kernel for nn_FNORCF1d — power-sum CFT restructure.

Sharding: data-parallel over batch (core b = batch element b). One tiny
AllGather (15us, vs 28us AllReduce) + local GpSimd partition-max per layer
for the CFT per-segment min/max.

Key restructurings vs the original kernel (sim: 748us -> 275us):
  - CFT via raw power sums: rec[c,s] = tanh(sum_k e_k[c,s] * S_k[c,s]) where
    S_k = mean((a h + b)^k) = (1/LS) sum_j C(k,j) a^j b^{k-j} P_j and
    P_j = sum_l h^j over the segment. P_j depends only on h, so the heavy
    elementwise phase is independent of the collective; only a tiny per-(c,s)
    binomial combine (a 36x36 matmul whose operand is built from b-powers on
    per-partition scalars) waits on it. Combine+gate-bias run per segment.
  - h, xfno, and the conv/gate/fc1 matmuls carried in bf16 (~6e-3 end-to-end,
    budget 2e-2); h -> hT via per-segment XBAR DMA block-transposes (no PE
    transposes, no PSUM->SBUF copies).
  - min/max read bf16 h directly via tensor_scalar(op1=max, accum_out) at the
    DVE 4x rate; products as tensor_tensor (2x) + tensor_scalar-accum sums
    (4x) -- scalar_tensor_tensor/tensor_reduce have no DVE perf modes.
  - engine split: squares on Act, odd products+sums+minmax+updates on DVE,
    p6/p8 products on GpSimd (no STT/PSUM/X-reduce allowed there).
  - P1 rides the gate-update STT accumulators; gate at 1024-wide pairs.
  - tail gelu via erf (same act-table set as sigmoid: no table-load ping-pong
    while the last layer's sigmoids drain) + a sigmoid-set preload dummy.
  - xt packed [32, 512] (one 64KB f32r DMA, K=32 zero-masked fc0 stationary);
    y via z-stationary matmuls -> [128, 64] on partitions -> one DMA.
"""

import functools
import math
import os
from contextlib import ExitStack

import numpy as np
from ml_dtypes import bfloat16

import concourse.bass as bass
import concourse.bacc as bacc
import concourse.mybir as mybir
import concourse.tile as tile
from concourse.bass_utils import run_bass_kernel_spmd

F32 = mybir.dt.float32
F32R = mybir.dt.float32r
BF16 = mybir.dt.bfloat16
AF = mybir.ActivationFunctionType
ALU = mybir.AluOpType
AX = mybir.AxisListType

B, L, W, MODES, NL, SEG, CM, H1 = 8, 8192, 128, 32, 4, 4, 8, 128
LS = L // SEG           # 2048
NJ = L // 512           # 16 chunks of 512
NC128 = L // 128        # 64 chunks of 128
K2 = 2 * MODES          # 64 interleaved (re, im) rows
NP = 9                  # powers 0..8


def _cheb2poly():
    """tmat[n, k]: T_n(x) = sum_k tmat[n,k] x^k, n,k in 0..8."""
    t = np.zeros((NP, NP))
    t[0, 0] = 1.0
    t[1, 1] = 1.0
    for n in range(2, NP):
        t[n, 1:] += 2.0 * t[n - 1, :-1]
        t[n, :] -= t[n - 2, :]
    return t


def _host_consts():
    lg = np.arange(L)
    kg = np.arange(MODES)
    ang = 2.0 * np.pi * np.outer(lg, kg) / L          # [L, 32]
    fall = np.zeros((L, 128), np.float32)
    fall[:, :MODES] = np.cos(ang)
    fall[:, MODES:K2] = -np.sin(ang)
    fcat = fall.reshape(NC128, 128, 128).transpose(1, 0, 2).reshape(128, NC128 * 128)
    coefk = (np.where(kg == 0, 1.0, 2.0) / L).astype(np.float32)
    minv = np.zeros((128, L), np.float32)
    minv[0:K2:2] = coefk[:, None] * np.cos(ang.T)
    minv[1:K2:2] = -coefk[:, None] * np.sin(ang.T)

    # binomial diagonal masks: block d has C(j+d, j) at (row s*9+j+d, col s*9+j)
    hcd = np.zeros((128, NP * 128), np.float32)
    for d in range(NP):
        for s in range(SEG):
            for j in range(NP - d):
                hcd[s * NP + j + d, d * 128 + s * NP + j] = math.comb(j + d, j)
    return {
        "fcat": fcat.astype(bfloat16),
        "minv": minv.astype(bfloat16),
        "ident_f": np.eye(128, dtype=np.float32),
        "ident_b": np.eye(128).astype(bfloat16),
        "ones128": np.ones((128, 128), np.float32),
        "hcd": hcd.astype(bfloat16),
    }


def _host_weights(inputs):
    spec = np.empty((NL, 128, MODES * 256), np.float32)
    for i in range(NL):
        for k in range(MODES):
            spec[i, :, k * 256: k * 256 + 128] = inputs["spec_wr"][i][:, :, k]
            spec[i, :, k * 256 + 128: (k + 1) * 256] = inputs["spec_wi"][i][:, :, k]

    # e-fold: arg[c,s] = sum_k e_k[c,s] S_k[c,s]
    # coeffs C_m = mean(T1*T_m) = mean(T_{m+1} + T_{|m-1|})/2; d_n = weights on
    # mean(T_n); e_k = sum_n d_n tmat[n,k] / LS  (1/LS folded in).
    tmat = _cheb2poly()
    cb = inputs["cheb_w"].mean(-1)                    # [NL, SEG, CM, W]
    d = np.zeros((NL, SEG, NP, W))
    d[:, :, 1] += cb[:, :, 0]
    for m in range(1, CM):
        d[:, :, m + 1] += cb[:, :, m] / 2
        d[:, :, abs(m - 1)] += cb[:, :, m] / 2
    e = np.einsum('lsnc,nk->lskc', d, tmat) / LS      # [NL, SEG, 9, W]
    eT = np.zeros((128, NL * 128), np.float32)        # rows (s*9+k), cols l*128+c
    for li in range(NL):
        for s in range(SEG):
            for k in range(NP):
                eT[s * NP + k, li * 128: (li + 1) * 128] = e[li, s, k, :]

    fc0w2 = np.zeros((2 * NJ, NJ * W), np.float32)  # block j: rows 2j:2j+2
    for j in range(NJ):
        fc0w2[2 * j:2 * j + 2, j * W:(j + 1) * W] = inputs["fc0_w"]
    fc2wS = inputs["fc2_w"].astype(np.float32)        # [128, 1]
    fc2bS = np.full((128, 1), float(inputs["fc2_b"][0]), np.float32)

    return {
        "spec": spec.astype(bfloat16),
        "convw": np.concatenate(list(inputs["conv_w"]), axis=1).astype(bfloat16),
        "convb": inputs["conv_b"].T.astype(np.float32),
        "gwa": np.concatenate([inputs["gate_w"][i][:W] for i in range(NL)], axis=1).astype(bfloat16),
        "gwb": np.concatenate([inputs["gate_w"][i][W:] for i in range(NL)], axis=1).astype(np.float32),
        "gateb": inputs["gate_b"].T.astype(np.float32),
        "eT": eT,
        "fc0w2": fc0w2,
        "fc0b": inputs["fc0_b"].reshape(W, 1).astype(np.float32),
        "fc1w": inputs["fc1_w"].astype(bfloat16),
        "fc1b": (inputs["fc1_b"] / np.sqrt(2.0)).reshape(H1, 1).astype(np.float32),
        "fc2wS": fc2wS,
        "fc2bS": fc2bS,
    }


_SPECS = {
    "xt": ((2 * NJ, 512), F32R),
    "fcat": ((128, NC128 * 128), BF16),
    "minv": ((128, L), BF16),
    "ident_f": ((128, 128), F32),
    "ident_b": ((128, 128), BF16),
    "ones128": ((128, 128), F32),
    "hcd": ((128, NP * 128), BF16),
    "spec": ((NL, 128, MODES * 256), BF16),
    "convw": ((128, NL * 128), BF16),
    "convb": ((128, NL), F32),
    "gwa": ((128, NL * 128), BF16),
    "gwb": ((128, NL * 128), F32),
    "gateb": ((128, NL), F32),
    "eT": ((128, NL * 128), F32),
    "fc0w2": ((2 * NJ, NJ * W), F32R),
    "fc0b": ((128, 1), F32),
    "fc1w": ((128, 128), BF16),
    "fc1b": ((128, 1), F32),
    "fc2wS": ((128, 1), F32),
    "fc2bS": ((128, 1), F32),
}

# load order matters: first items unblock fc0 + layer-0 FNO
_CONST_NAMES = ["fc0w2", "fc0b", "ident_f", "fcat", "ident_b", "minv",
                "convw", "convb", "hcd", "eT", "ones128", "gwa", "gwb", "gateb",
                "fc1w", "fc1b", "fc2wS", "fc2bS"]
_CONST_EARLY = ["fc0w2", "fc0b", "ident_f", "fcat", "ident_b"]


def _emit(tc, ap, ctx):
    lvl = int(os.environ.get("KBISECT", "99"))
    nc = tc.nc
    ec = ctx.enter_context

    cpool = ec(tc.tile_pool(name="const", bufs=1))
    spool = ec(tc.tile_pool(name="spec", bufs=2))
    state = ec(tc.tile_pool(name="state", bufs=1))
    cft = ec(tc.tile_pool(name="cft", bufs=2))
    small = ec(tc.tile_pool(name="small", bufs=2))
    gpool = ec(tc.tile_pool(name="gate", bufs=2))
    dpool = ec(tc.tile_pool(name="dram", bufs=4, space="DRAM"))
    ptr = ec(tc.tile_pool(name="ptr", bufs=2, space="PSUM"))
    psmall = ec(tc.tile_pool(name="psmall", bufs=2, space="PSUM"))
    pbig = ec(tc.tile_pool(name="pbig", bufs=2, space="PSUM"))

    tiles = {}
    xt32_holder = []

    def _load(names):
        for name in names:
            sh, dt_ = _SPECS[name]
            t = cpool.tile(list(sh), dt_, tag=name, name=f"c_{name}")
            nc.sync.dma_start(t[:], ap[name][:])
            tiles[name] = t
    xt32_pre = small.tile([2 * NJ, 512], F32R, tag="xt32", bufs=1)
    nc.sync.dma_start(xt32_pre[:], ap["xt"][:])
    xt32_holder.append(xt32_pre)
    _load(_CONST_EARLY)

    h = state.tile([128, L], BF16, tag="h")
    xfno = state.tile([128, L], BF16, tag="xfno")
    hT = state.tile([128, L], BF16, tag="hT")
    accP = state.tile([128, 128], F32, tag="accP")       # cols (s*9+j), j=0..8
    accP1c2 = state.tile([128, NJ // 2], F32, tag="accP1c")  # gate-part P1
    accP1g = state.tile([128, NJ // 2], F32, tag="accP1g")  # fc0 P1
    arow = state.tile([128, SEG * NP], F32, tag="arow")  # row0: a^j at (s*9+j)
    brow = state.tile([128, 128], F32, tag="brow")       # row0: b at (s*9+k)
    bdcol = state.tile([128, NP], F32, tag="bdcol")      # col d: b^d rows (s,k)
    mbt = state.tile([128, 128], F32, tag="mbt")
    qTsb = state.tile([128, 128], F32, tag="qTsb")
    omT_sb = state.tile([128, 128], BF16, tag="omT_sb")

    # one-time zero/const initialization
    nc.gpsimd.memset(arow[:], 0.0)
    nc.gpsimd.memset(brow[:], 0.0)
    nc.gpsimd.memset(omT_sb[:], 0.0)
    nc.gpsimd.memset(bdcol[:], 1.0)          # col 0 stays 1; cols 1.. rebuilt
    nc.gpsimd.memset(accP[:], float(LS))     # (s,0) cols stay LS; rest rebuilt
    arv = arow[0:1, 0:SEG * NP].rearrange("one (s j) -> one s j", j=NP)
    nc.gpsimd.memset(arv[:, :, 0:1], 1.0)    # a^0 = 1

    spec_t = [None] * NL
    spec_t[0] = spool.tile([128, MODES * 256], BF16, tag="spec", name="spec0")
    nc.sync.dma_start(spec_t[0][:], ap["spec"][0])
    _load([n for n in _CONST_NAMES if n not in _CONST_EARLY])
    fcat, minv = tiles["fcat"], tiles["minv"]
    ident_f, ident_b = tiles["ident_f"], tiles["ident_b"]
    ones128, hcd, eT = tiles["ones128"], tiles["hcd"], tiles["eT"]
    convw, convb = tiles["convw"], tiles["convb"]
    gwa, gwb, gateb = tiles["gwa"], tiles["gwb"], tiles["gateb"]
    fc0w2, fc0b = tiles["fc0w2"], tiles["fc0b"]
    fc1w, fc1b = tiles["fc1w"], tiles["fc1b"]
    fc2wS, fc2bS = tiles["fc2wS"], tiles["fc2bS"]


    # ---- fc0: h = fc0_w.T @ x (K=2 contraction per row pair) + P1 accum ----
    xt32 = xt32_holder[0]
    for j2 in range(NJ // 2):
        js = slice(j2 * 1024, (j2 + 1) * 1024)
        p = pbig.tile([128, 1024], F32, tag="xfp", bufs=1)
        for hh in range(2):
            j = 2 * j2 + hh
            nc.tensor.matmul(p[:, hh * 512:(hh + 1) * 512],
                             fc0w2[:, j * W:(j + 1) * W],
                             xt32[:], start=True, stop=True)
        nc.scalar.activation(h[:, js], p[:], AF.Identity, bias=fc0b[:],
                             accum_out=accP1g[:, j2:j2 + 1])

    nlayers = 0 if lvl < 2 else NL
    for i in range(nlayers):
        last = i == NL - 1
        # ---- h -> hT via per-segment DMA block-transposes (XBAR) ----
        for s in range(SEG):
            hTv = hT[:, s * LS:(s + 1) * LS].rearrange("p (c j) -> p c j", j=128)
            nc.sync.dma_start_transpose(hTv, h[:, s * LS:(s + 1) * LS])

        # ---- min/max direct from bf16 h segments (DVE 4x) -> AllGather ----
        catmm = small.tile([128, 8], F32, tag="catmm")
        with tc.high_priority():
            for s in range(SEG):
                hs = h[:, s * LS:(s + 1) * LS]
                mmscr = cft.tile([128, LS], BF16, tag="scr", bufs=2)
                nc.vector.tensor_scalar(mmscr[:], hs, -1.0, None, ALU.mult,
                                        ALU.max, accum_out=catmm[:, s:s + 1])
                nc.vector.tensor_scalar(mmscr[:], hs, 1.0, None, ALU.mult,
                                        ALU.max,
                                        accum_out=catmm[:, SEG + s:SEG + s + 1])
        redrow = small.tile([1, 2 * SEG], F32, tag="redrow")
        nc.gpsimd.tensor_reduce(redrow[:], catmm[:], AX.C, ALU.max)
        ccin = dpool.tile([1, 2 * SEG], F32, tag="ccin")
        ccout = dpool.tile([B, 2 * SEG], F32, tag="ccout")
        nc.gpsimd.dma_start(ccin[:], redrow[:])
        nc.gpsimd.collective_compute(
            "AllGather", ALU.bypass,
            ins=[ccin[:].opt()], outs=[ccout[:].opt()],
            replica_groups=[list(range(B))],
        )

        # ---- forward DFT + mode mix (PE; small copies on Pool) ----
        phfT = psmall.tile([128, 128], F32, tag="sm", bufs=2)
        for c in range(NC128):
            nc.tensor.matmul(phfT[:], fcat[:, c * 128:(c + 1) * 128],
                             hT[:, c * 128:(c + 1) * 128],
                             start=(c == 0), stop=(c == NC128 - 1))
        hfT_sb = small.tile([128, 128], BF16, tag="hfT_sb", bufs=1)
        nc.scalar.activation(hfT_sb[0:K2, :], phfT[0:K2, :], AF.Identity)
        phf = psmall.tile([128, 128], BF16, tag="sm", bufs=2)
        nc.tensor.transpose(phf[:], hfT_sb[:], ident_b[:])
        rhs1 = small.tile([128, K2], BF16, tag="rhs1", bufs=1)
        rhs2 = small.tile([128, K2], BF16, tag="rhs2", bufs=1)
        r1v = rhs1[:].rearrange("p (k two) -> p k two", two=2)
        r2v = rhs2[:].rearrange("p (k two) -> p k two", two=2)
        hrv = phf[:, 0:MODES].rearrange("p k -> p k ()")
        hiv = phf[:, MODES:K2].rearrange("p k -> p k ()")
        nc.vector.tensor_copy(r1v[:, :, 0:1], hrv)
        nc.vector.tensor_copy(r1v[:, :, 1:2], hiv)
        nc.vector.tensor_scalar(r2v[:, :, 0:1], hiv, -1.0, None, ALU.mult)
        nc.vector.tensor_copy(r2v[:, :, 1:2], hrv)

        pom = psmall.tile([128, K2], F32, tag="sm", bufs=2)
        for k in range(MODES):
            nc.tensor.matmul(pom[:, 2 * k:2 * k + 2],
                             spec_t[i][:, k * 256:k * 256 + 128],
                             rhs1[:, 2 * k:2 * k + 2], start=True, stop=False)
            nc.tensor.matmul(pom[:, 2 * k:2 * k + 2],
                             spec_t[i][:, k * 256 + 128:(k + 1) * 256],
                             rhs2[:, 2 * k:2 * k + 2], start=False, stop=True)
        om_sb = small.tile([128, 128], BF16, tag="om_sb", bufs=1)
        nc.scalar.activation(om_sb[:, 0:K2], pom[:], AF.Identity)
        pomT = psmall.tile([128, 128], BF16, tag="sm", bufs=2)
        nc.tensor.transpose(pomT[:], om_sb[:], ident_b[:])
        nc.scalar.activation(omT_sb[0:K2, :], pomT[0:K2, :], AF.Identity)

        if i + 1 < NL:
            spec_t[i + 1] = spool.tile([128, MODES * 256], BF16, tag="spec",
                                       name=f"spec{i + 1}")
            nc.sync.dma_start(spec_t[i + 1][:], ap["spec"][i + 1])

        if lvl < 4:
            continue
        # ---- power sums P_1..P_8 per segment ----
        aPv = accP[0:128, 0:SEG * NP].rearrange("p (s j) -> p s j", j=NP)
        if i == 0:
            a1v = accP1g[:].rearrange("p (s f) -> p s f", f=(NJ // 2) // SEG)
            nc.vector.tensor_reduce(aPv[:, :, 1:2], a1v, AX.X, ALU.add)
        else:
            a1c = accP1c2[:].rearrange("p (s f) -> p s f", f=(NJ // 2) // SEG)
            nc.vector.tensor_reduce(aPv[:, :, 1:2], a1c, AX.X, ALU.add)
        for s in range(SEG):
            hseg = h[:, s * LS:(s + 1) * LS]
            sj = s * NP
            h2 = cft.tile([128, LS], BF16, tag="h2", bufs=1)
            nc.scalar.activation(h2[:], hseg, AF.Square,
                                 accum_out=accP[:, sj + 2:sj + 3])
            h3 = cft.tile([128, LS], BF16, tag="h3", bufs=2)
            nc.vector.tensor_tensor(h3[:], h2[:], hseg, ALU.mult)
            h4 = cft.tile([128, LS], BF16, tag="h4", bufs=1)
            nc.scalar.activation(h4[:], h2[:], AF.Square,
                                 accum_out=accP[:, sj + 4:sj + 5])
            scr3 = cft.tile([128, LS], BF16, tag="scr", bufs=2)
            nc.vector.tensor_scalar(scr3[:], h3[:], 1.0, None, ALU.mult, ALU.add,
                                    accum_out=accP[:, sj + 3:sj + 4])
            scr5 = cft.tile([128, LS], BF16, tag="scr", bufs=2)
            nc.vector.tensor_tensor(scr5[:], h2[:], h3[:], ALU.mult)
            nc.vector.tensor_scalar(scr5[:], scr5[:], 1.0, None, ALU.mult, ALU.add,
                                    accum_out=accP[:, sj + 5:sj + 6])
            scr6 = cft.tile([128, LS], BF16, tag="scrp", bufs=1)
            nc.gpsimd.tensor_tensor(scr6[:], h3[:], h3[:], ALU.mult)
            nc.vector.tensor_scalar(scr6[:], scr6[:], 1.0, None, ALU.mult, ALU.add,
                                    accum_out=accP[:, sj + 6:sj + 7])
            scr7 = cft.tile([128, LS], BF16, tag="scr", bufs=2)
            nc.vector.tensor_tensor(scr7[:], h3[:], h4[:], ALU.mult)
            nc.vector.tensor_scalar(scr7[:], scr7[:], 1.0, None, ALU.mult, ALU.add,
                                    accum_out=accP[:, sj + 7:sj + 8])
            scr8 = cft.tile([128, LS], BF16, tag="scrp", bufs=1)
            nc.gpsimd.tensor_tensor(scr8[:], h4[:], h4[:], ALU.mult)
            nc.vector.tensor_scalar(scr8[:], scr8[:], 1.0, None, ALU.mult, ALU.add,
                                    accum_out=accP[:, sj + 8:sj + 9])

        if lvl < 5:
            continue
        # ---- spectral + conv -> gelu -> x_fno ----
        for j2 in range(NJ // 2):
            js = slice(j2 * 1024, (j2 + 1) * 1024)
            p = pbig.tile([128, 1024], F32, tag="xfp", bufs=1)
            for hh in range(2):
                hs = slice(j2 * 1024 + hh * 512, j2 * 1024 + (hh + 1) * 512)
                nc.tensor.matmul(p[:, hh * 512:(hh + 1) * 512], omT_sb[:],
                                 minv[:, hs], start=True, stop=False)
                nc.tensor.matmul(p[:, hh * 512:(hh + 1) * 512],
                                 convw[:, i * 128:(i + 1) * 128], h[:, hs],
                                 start=False, stop=True)
            nc.scalar.activation(xfno[:, js], p[:], AF.Gelu, bias=convb[:, i:i + 1])

        sgw = small.tile([1, 1], F32, tag="sgw")
        nc.scalar.activation(sgw[:], xfno[0:1, L - 1:L], AF.Sigmoid)

        if lvl < 6:
            continue
        # ---- combine (Pool smalls wait on the AllGather) ----
        gat = small.tile([B, 2 * SEG], F32, tag="gat")
        abrow = small.tile([1, 2 * SEG], F32, tag="abrow")
        with tc.high_priority():
            nc.sync.dma_start(gat[:], ccout[:])
            nc.gpsimd.tensor_reduce(abrow[:], gat[:], AX.C, ALU.max)
        hp = tc.high_priority()
        hp.__enter__()
        negd = small.tile([1, SEG], F32, tag="negd")
        nc.gpsimd.tensor_tensor(negd[:], abrow[:, 0:SEG], abrow[:, SEG:2 * SEG], ALU.add)
        inv = small.tile([1, SEG], F32, tag="invd")
        nc.vector.reciprocal(inv[:], negd[:])          # 1/(mx-mn)
        a4 = small.tile([1, SEG], F32, tag="a4")
        nc.gpsimd.tensor_scalar(a4[:], inv[:], 2.0, None, ALU.mult)
        m1 = small.tile([1, SEG], F32, tag="m1")
        nc.gpsimd.tensor_tensor(m1[:], abrow[:, 0:SEG], inv[:], ALU.mult)
        b4 = small.tile([1, SEG], F32, tag="b4")
        nc.gpsimd.tensor_scalar(b4[:], m1[:], 2.0, -1.0, ALU.mult, ALU.add)
        # arow row0: a^j; brow row0: b at every (s,k)
        for j in range(1, NP):
            nc.gpsimd.tensor_tensor(arv[:, :, j:j + 1], arv[:, :, j - 1:j],
                                    a4[:].rearrange("one s -> one s ()"), ALU.mult)
        brv = brow[0:1, 0:SEG * NP].rearrange("one (s k) -> one s k", k=NP)
        for k in range(NP):
            nc.gpsimd.tensor_copy(brv[:, :, k:k + 1],
                                  b4[:].rearrange("one s -> one s ()"))
        pcol = psmall.tile([128, 128], F32, tag="sm", bufs=2)
        nc.tensor.transpose(pcol[:], brow[:], ident_f[:])
        bcol = small.tile([128, 1], F32, tag="bcol")
        nc.vector.tensor_copy(bcol[:], pcol[:, 0:1])
        nc.gpsimd.tensor_copy(bdcol[:, 1:2], bcol[:])
        nc.gpsimd.tensor_tensor(bdcol[:, 2:3], bcol[:], bcol[:], ALU.mult)
        nc.gpsimd.tensor_tensor(bdcol[:, 3:4], bdcol[:, 1:2], bdcol[:, 2:3], ALU.mult)
        nc.gpsimd.tensor_tensor(bdcol[:, 4:5], bdcol[:, 2:3], bdcol[:, 2:3], ALU.mult)
        nc.gpsimd.tensor_tensor(bdcol[:, 5:6], bdcol[:, 2:3], bdcol[:, 3:4], ALU.mult)
        nc.gpsimd.tensor_tensor(bdcol[:, 6:7], bdcol[:, 3:4], bdcol[:, 3:4], ALU.mult)
        nc.gpsimd.tensor_tensor(bdcol[:, 7:8], bdcol[:, 3:4], bdcol[:, 4:5], ALU.mult)
        nc.gpsimd.tensor_tensor(bdcol[:, 8:9], bdcol[:, 4:5], bdcol[:, 4:5], ALU.mult)
        mb2 = small.tile([128, 128], F32, tag="mb2", bufs=1)
        mb3 = small.tile([128, 128], F32, tag="mb3", bufs=1)
        nc.gpsimd.tensor_copy(mbt[:], hcd[:, 0:128])
        for d in (1, 2):
            nc.vector.scalar_tensor_tensor(mbt[:], hcd[:, d * 128:(d + 1) * 128],
                                           bdcol[:, d:d + 1], mbt[:],
                                           ALU.mult, ALU.add)
        nc.vector.tensor_scalar(mb2[:], hcd[:, 3 * 128:4 * 128], bdcol[:, 3:4],
                                None, ALU.mult)
        for d in (4, 5):
            nc.vector.scalar_tensor_tensor(mb2[:], hcd[:, d * 128:(d + 1) * 128],
                                           bdcol[:, d:d + 1], mb2[:],
                                           ALU.mult, ALU.add)
        nc.vector.tensor_scalar(mb3[:], hcd[:, 6 * 128:7 * 128], bdcol[:, 6:7],
                                None, ALU.mult)
        for d in (7, 8):
            nc.vector.scalar_tensor_tensor(mb3[:], hcd[:, d * 128:(d + 1) * 128],
                                           bdcol[:, d:d + 1], mb3[:],
                                           ALU.mult, ALU.add)
        nc.vector.tensor_tensor(mb2[:], mb2[:], mb3[:], ALU.add)
        nc.vector.tensor_tensor(mbt[:], mbt[:], mb2[:], ALU.add)
        pqT = psmall.tile([128, 128], F32, tag="sm", bufs=2)
        nc.tensor.matmul(pqT[:], mbt[:], eT[:, i * 128:(i + 1) * 128],
                         start=True, stop=True)
        nc.scalar.activation(qTsb[:], pqT[:], AF.Identity)
        pQ = psmall.tile([128, 128], F32, tag="sm", bufs=2)
        nc.tensor.transpose(pQ[:], qTsb[:], ident_f[:])
        pA = psmall.tile([128, SEG * NP], F32, tag="sm", bufs=2)
        nc.tensor.matmul(pA[:], ones128[:], arow[:, 0:SEG * NP],
                         start=True, stop=True)
        # per-segment combine tail: rec[s] only needs segment-s power sums,
        # so early segments' gate pairs start while later products finish
        u = small.tile([128, SEG * NP], F32, tag="u")
        parg = small.tile([128, SEG], F32, tag="parg")
        rec = small.tile([128, SEG], F32, tag="rec")
        biasg = small.tile([128, SEG], F32, tag="biasg")
        for s in range(SEG):
            sl = slice(s * NP, (s + 1) * NP)
            nc.vector.tensor_tensor(u[:, sl], accP[:, sl], pA[:, sl], ALU.mult)
            nc.vector.tensor_tensor(u[:, sl], u[:, sl], pQ[:, sl], ALU.mult)
            nc.vector.tensor_reduce(
                parg[:, s:s + 1],
                u[:, sl].rearrange("p (one j) -> p one j", one=1), AX.X, ALU.add)
            nc.scalar.activation(rec[:, s:s + 1], parg[:, s:s + 1], AF.Tanh)
            pgs = psmall.tile([128, 1], F32, tag="sm", bufs=2, name=f"pgs{i}_{s}")
            nc.tensor.matmul(pgs[:], gwb[:, i * 128:(i + 1) * 128], rec[:, s:s + 1],
                             start=True, stop=True)
            nc.vector.tensor_scalar(biasg[:, s:s + 1], pgs[:], gateb[:, i:i + 1],
                                    None, ALU.add)
        hp.__exit__(None, None, None)

        if lvl < 7:
            continue

        for j2 in range(NJ // 2):
            js = slice(j2 * 1024, (j2 + 1) * 1024)
            s = j2 // 2
            pg = pbig.tile([128, 1024], F32, tag="gate", bufs=2)
            nc.tensor.matmul(pg[:, 0:512], gwa[:, i * 128:(i + 1) * 128],
                             xfno[:, j2 * 1024:j2 * 1024 + 512], start=True, stop=True)
            nc.tensor.matmul(pg[:, 512:1024], gwa[:, i * 128:(i + 1) * 128],
                             xfno[:, j2 * 1024 + 512:(j2 + 1) * 1024],
                             start=True, stop=True)
            gchunk = gpool.tile([128, 1024], BF16, tag="g")
            nc.scalar.activation(gchunk[:], pg[:], AF.Sigmoid, bias=biasg[:, s:s + 1])
            nc.vector.scalar_tensor_tensor(
                h[:, js], gchunk[:], rec[:, s:s + 1], xfno[:, js],
                ALU.mult, ALU.add,
                accum_out=None if last else accP1c2[:, j2:j2 + 1])

    if lvl < 9:
        return
    # ---- tail: fc1 -> gelu -> fc2 -> y ----
    # gelu via erf (stays in the sigmoid act-table set: no table reloads while
    # the last layer's sigmoids drain): gelu(x) = (0.5 erf(x/sqrt2) + 0.5) * x
    z = state.tile([128, L], F32, tag="ztail")
    for j2 in range(NJ // 2):
        js = slice(j2 * 1024, (j2 + 1) * 1024)
        p = pbig.tile([128, 1024], F32, tag="gate", bufs=2)
        nc.tensor.matmul(p[:, 0:512], fc1w[:], h[:, j2 * 1024:j2 * 1024 + 512],
                         start=True, stop=True)
        nc.tensor.matmul(p[:, 512:1024], fc1w[:],
                         h[:, j2 * 1024 + 512:(j2 + 1) * 1024],
                         start=True, stop=True)
        ec_ = gpool.tile([128, 1024], F32, tag="gr", bufs=1)
        nc.scalar.activation(ec_[:], p[:], AF.Erf, bias=fc1b[:],
                             scale=float(1.0 / np.sqrt(2.0)))
        nc.vector.tensor_scalar(ec_[:], ec_[:], 0.5, 0.5, ALU.mult, ALU.add)
        nc.vector.tensor_tensor(z[:, js], ec_[:], p[:], ALU.mult)
    # y on partitions: out[p, c] = sum_k z[k, c*128+p] fc2w[k] -> y[c*128+p]
    py = psmall.tile([128, 128], F32, tag="sm", bufs=2)
    yc = small.tile([128, NC128], F32, tag="yc", bufs=1)
    for g in range(NC128 // 32):
        for t in range(32):
            c = g * 32 + t
            nc.tensor.matmul(py[:, t:t + 1], z[:, c * 128:(c + 1) * 128],
                             fc2wS[:, 0:1], start=True, stop=True)
        nc.scalar.activation(yc[:, g * 32:(g + 1) * 32], py[:, 0:32],
                             AF.Identity, bias=fc2bS[:])
    nc.sync.dma_start(ap["y"][:], yc[:])


@functools.lru_cache(maxsize=1)
def _build():
    nc = bacc.Bacc("TRN2", target_bir_lowering=False, debug=False, num_devices=B)
    ap = {}
    for name, (shape, dt_) in _SPECS.items():
        ap[name] = nc.dram_tensor(name, list(shape), dt_, kind="ExternalInput").ap()
    ap["y"] = nc.dram_tensor("y", [128, NC128], F32, kind="ExternalOutput").ap()
    with tile.TileContext(nc) as tc:
        with ExitStack() as ctx:
            _emit(tc, ap, ctx)
    nc.compile()
    return nc


def kernel(**inputs):
    inputs = {k: np.asarray(v) for k, v in inputs.items()}
    consts = _host_consts()
    weights = _host_weights(inputs)
    nc = _build()

    shared = {}
    for name in _SPECS:
        if name == "xt":
            continue
        src = consts.get(name)
        if src is None:
            src = weights[name]
        shared[name] = np.ascontiguousarray(src)
    in_maps = []
    for b in range(B):
        m = dict(shared)
        xv = inputs["x"][b].T.astype(np.float32).reshape(2, NJ, 512)
        m["xt"] = np.ascontiguousarray(xv.transpose(1, 0, 2).reshape(2 * NJ, 512))
        in_maps.append(m)

    res = run_bass_kernel_spmd(nc, in_maps, list(range(B)))
    out = np.stack([res.results[b]["y"].T.reshape(L, 1) for b in range(B)])
    return out.astype(np.float32)


if __name__ == "__main__":
    import reference

    inputs = reference.setup_inputs()
    out = kernel(**{k: np.asarray(v) for k, v in inputs.items()})
    print(out.shape, np.abs(out).max())
